# revision 18
# baseline (speedup 1.0000x reference)
"""GCNN message-passing layer on 8 Trainium2 NeuronCores (Bass/Tile).

Math (per token m, all within one sentence of L=64 tokens):
    in_pot[m]  = (rep @ W_in)[head(m)] + b_in[lab(m)]
    in_gate[m] = (rep @ W_gate_in)[head(m)] + b_gate_in[lab(m)]
    self_pot   = rep @ W_self ; self_gate = rep @ W_gate_self
    w_d = sigmoid(gate_d) * msoft_d^2
    out = relu(in_pot*w_in + self_pot*w_self) * mask

Sharding: data-parallel over BNK (160 sentences / core). All gathers stay
within a sentence, so shards are independent; weights are replicated.

Device strategy per 128-token tile (2 sentences):
  - The gate paths (rep @ W_gate_*, 0.2% of the FLOPs) run on the host;
    sigmoid(gate)*msoft^2*mask folds into the one-hot scatter values (w_in
    side) and into a per-token aux vector (w_self side). The device never
    computes gates, sigmoids, or masks.
  - rep arrives host-pretransposed (fp16) so DIN sits on partitions. One
    512-column moving operand [W_in | W_self] turns the two projections
    into 4 accumulating matmuls per tile (512-cycle streams fully hide
    each LDWEIGHTS).
  - The within-tile head gather is a matmul with a host-built one-hot
    scatter matrix whose nonzeros carry w_in; it is software-pipelined one
    tile behind the projections so the in-order tensor queue never waits
    on the PSUM->fp16 cast feeding it. (A relation-bias matmul joins the
    same accumulation only when b_in != 0; setup_inputs has b_in == 0.)
  - Tail per tile: one ACT op casts the whole [in_pot|self_pot] PSUM bank
    to fp16 (gather src + self operand), then one fused custom-DVE op
    emits relu(in_pot_gathered + w_self*self_pot) straight to fp16.
  - Output stays partition-major in DRAM ([128, ntiles, dout]) so the DMA
    moves 4KB-contiguous runs; the host de-interleaves.
  - Startup: ~32 throwaway matmuls release the PE HAM clock gate while
    the first DMAs land; wcat rides the Scalar HWDGE queue concurrently
    with rep on the SP queue, and the first four tiles' rep arrives as
    single-tile DMAs, so real matmuls start ~9 us in, still warm.
  - Outputs ride the GpSimd SWDGE queue except the last batch, which goes
    on the Scalar HWDGE queue so the end-of-kernel SWDGE drain finds an
    idle queue.
"""

import numpy as np

import concourse.bass as bass
import concourse.dve_ops as dve_ops
import concourse.mybir as mybir
import concourse.tile as tile
from concourse import bacc, bass_utils
from concourse.dve_spec import C0, C1, Spec, Src0, Src1, lower as dve_lower, relu as dve_relu
from concourse.dve_uop import DveOpSpec


def _register_gated_relu_op():
    """Register a fused custom-DVE op: out = relu(in0*s0 + in1*s1)."""
    name = "GCNN_GATED_RELU_ANT"
    for op in dve_ops.OPS:
        if op.name == name:
            return op
    spec = Spec(
        body=dve_relu(Src0 * C0 + Src1 * C1),
        reference=lambda in0, in1, s0, s1, imm2: np.maximum(
            np.nan_to_num(in0.astype(np.float32) * s0 + in1 * s1,
                          nan=0.0, posinf=np.inf, neginf=-np.inf), 0.0),
    )
    row = dve_ops._CUSTOM_DVE_ROW_BASE + len(dve_ops.OPS)
    dve_ops._SUB_OPCODE_FOR_NAME[name] = row
    shas = {}
    for ver in ("v3", "v4"):
        uops = dve_lower(spec, ver=ver)
        shas[ver] = DveOpSpec(name=name, opcode=row, uops=uops, rd1_en=True).sha(ver)
    op = dve_ops.DveOp(name, spec, subdim=False, uops_sha=shas)
    dve_ops.OPS.append(op)
    dve_ops.CUSTOM_DVE_SPECS[name] = spec
    return op


GATED_RELU = _register_gated_relu_op()

BNK, L, DIN, DOUT, NREL = 1280, 64, 512, 256, 40
NCORES = 8
SPC = BNK // NCORES          # sentences per core
TOK = SPC * L                # tokens per core (10240)
TILE_T = 128                 # tokens per device tile
KC = DIN // 128              # K chunks (4)
NTILES = TOK // TILE_T       # 80
OGROUP = 4                   # tiles per output DMA batch
NWARM = 32                   # HAM warmup matmuls

F32 = mybir.dt.float32
F16 = mybir.dt.float16
NP_MM = np.float16
AF = mybir.ActivationFunctionType


def _in_groups(ntiles):
    """Input DMA batching: single tiles first (fast start), then fours."""
    gs = [(0, 1), (1, 1), (2, 1), (3, 1)]
    i = 4
    while i < ntiles:
        sz = min(4, ntiles - i)
        gs.append((i, sz))
        i += sz
    return gs


def build_nc(ntiles: int = NTILES, lab_bias: bool = False):
    """Build the per-core Bass program (same program on all cores).

    lab_bias=True adds the relation-bias matmul (needed only when b_in is
    nonzero; setup_inputs always produces b_in == 0).
    """
    assert ntiles % OGROUP == 0
    nc = bacc.Bacc("TRN2", target_bir_lowering=False, debug=False)

    # --- DRAM I/O (flat, partition-major; sliced per DMA batch) ---------
    repT_d = nc.dram_tensor("repT", [128, ntiles, KC, TILE_T], F16, kind="ExternalInput")
    scatW_d = nc.dram_tensor("scatW", [TILE_T, ntiles, TILE_T], F16, kind="ExternalInput")
    if lab_bias:
        scatL_d = nc.dram_tensor("scatL", [NREL, ntiles, TILE_T], F16, kind="ExternalInput")
        ball_d = nc.dram_tensor("ball", [NREL, DOUT], F16, kind="ExternalInput")
    wcat_d = nc.dram_tensor("wcat", [128, KC, 2 * DOUT], F16, kind="ExternalInput")
    # aux[:, i] = w_self for tile i; last column is the constant 1.0
    aux_d = nc.dram_tensor("aux", [128, ntiles + 1], F32, kind="ExternalInput")
    # partition-major output: [p, tile, dout]; host de-interleaves
    out_d = nc.dram_tensor("out", [TILE_T, ntiles, DOUT], F16, kind="ExternalOutput")

    groups = _in_groups(ntiles)
    with tile.TileContext(nc) as tc:
        with (
            tc.tile_pool(name="const", bufs=1) as const_pool,
            tc.tile_pool(name="rep", bufs=6) as rep_pool,
            tc.tile_pool(name="scat", bufs=5) as scat_pool,
            tc.tile_pool(name="src", bufs=3) as src_pool,
            tc.tile_pool(name="out", bufs=3) as out_pool,
            tc.tile_pool(name="psum", bufs=3, space="PSUM") as psum_pool,
            tc.tile_pool(name="psum2", bufs=3, space="PSUM") as psum2_pool,
            tc.tile_pool(name="psumw", bufs=1, space="PSUM") as psumw_pool,
        ):
            # --- PE warmup: release the HAM clock gate while DMAs land ---
            wz = const_pool.tile([128, 16], F16)
            nc.gpsimd.memset(wz[:], 0.0)
            wp = psumw_pool.tile([16, 16], F32, tag="warm")
            for _ in range(NWARM):
                nc.tensor.matmul(wp[:], wz[:, 0:16], wz[:], start=True, stop=True)

            # wcat on the Scalar HWDGE queue, concurrent with rep on SP;
            # two kc-pair slices so the first matmuls wait on 256KB, not
            # 512KB, without serializing four trigger issues
            wcat_sb = [const_pool.tile([128, 2, 2 * DOUT], F16, tag=f"wcat{h}",
                                       name=f"wcat{h}")
                       for h in range(2)]
            for h in range(2):
                nc.scalar.dma_start(wcat_sb[h][:], wcat_d[:, 2 * h:2 * h + 2, :])
            aux_sb = const_pool.tile([128, ntiles + 1], F32)
            ones = aux_sb[:, ntiles:ntiles + 1]
            ball_sb = const_pool.tile([NREL, DOUT], F16) if lab_bias else None

            pend = None          # (i, src, o_sb, oslot, scat_sb, sslot, scatl_sb)
            pend_out = None      # (ostart, o_sb) awaiting its batched output DMA

            def flush_tail():
                nonlocal pend, pend_out
                if pend is None:
                    return
                i, src, o_sb, oslot, scat_sb, sslot, scatl_sb = pend
                psum_g = psum2_pool.tile([128, DOUT], F32, tag="pg")
                nc.tensor.matmul(psum_g[:], scat_sb[:, sslot, :], src[:, 0:DOUT],
                                 start=True, stop=not lab_bias)
                if lab_bias:
                    nc.tensor.matmul(psum_g[:], scatl_sb[:, sslot, :], ball_sb[:],
                                     start=False, stop=True)
                nc.vector._custom_dve(GATED_RELU, out=o_sb[:, oslot, :],
                                      in0=psum_g[:], in1=src[:, DOUT:2 * DOUT],
                                      s0=ones, s1=aux_sb[:, i:i + 1])
                pend = None
                if oslot == OGROUP - 1:
                    ostart, osb = pend_out
                    eng = nc.scalar if ostart + OGROUP >= ntiles else nc.gpsimd
                    eng.dma_start(out_d[:, ostart:ostart + OGROUP, :], osb[:])
                    pend_out = None

            o_sb = None
            for gi, (i0, sz) in enumerate(groups):
                rep_sb = rep_pool.tile([128, sz, KC, TILE_T], F16, tag="rep")
                nc.sync.dma_start(rep_sb[:], repT_d[:, i0:i0 + sz, :, :])
                if i0 == 0:
                    # tiles 0-3 share one scatter batch; allocate now (tile 0's
                    # tail references it) but DMA after rep1 so the queue
                    # delivers bytes in consumption order
                    scat_sb = scat_pool.tile([TILE_T, 4, TILE_T], F16, tag="scath")
                    s0 = 0
                    if lab_bias:
                        scatl_sb = scat_pool.tile([NREL, 4, TILE_T], F16, tag="scatl")
                elif i0 == 1:
                    # emitted before flush_tail(tile 0), which reads these
                    nc.sync.dma_start(scat_sb[:], scatW_d[:, 0:4, :])
                    nc.sync.dma_start(aux_sb[:], aux_d[:])
                    if lab_bias:
                        nc.sync.dma_start(scatl_sb[:], scatL_d[:, 0:4, :])
                        nc.sync.dma_start(ball_sb[:], ball_d[:])
                elif i0 >= 4:
                    scat_sb = scat_pool.tile([TILE_T, sz, TILE_T], F16, tag="scath")
                    nc.sync.dma_start(scat_sb[:], scatW_d[:, i0:i0 + sz, :])
                    s0 = i0
                    if lab_bias:
                        scatl_sb = scat_pool.tile([NREL, sz, TILE_T], F16, tag="scatl")
                        nc.sync.dma_start(scatl_sb[:], scatL_d[:, i0:i0 + sz, :])

                for ti in range(sz):
                    i = i0 + ti
                    if i % OGROUP == 0:
                        o_sb = out_pool.tile([128, OGROUP, DOUT], F16)
                    # [in_pot | self_pot] in one PSUM bank via a fused
                    # 512-column moving operand
                    psum_ab = psum_pool.tile([128, 2 * DOUT], F32, tag="pab")
                    for kc in range(KC):
                        nc.tensor.matmul(psum_ab[:], rep_sb[:, ti, kc, :],
                                         wcat_sb[kc // 2][:, kc % 2, :],
                                         start=kc == 0, stop=kc == KC - 1)
                    src = src_pool.tile([128, 2 * DOUT], F16)
                    nc.scalar.activation(src[:], psum_ab[:], AF.Copy)
                    flush_tail()
                    if i % OGROUP == OGROUP - 1:
                        pend_out = (i - OGROUP + 1, o_sb)
                    pend = (i, src, o_sb, i % OGROUP, scat_sb, i - s0,
                            scatl_sb if lab_bias else None)
            flush_tail()

    nc.compile()
    return nc


def _sigmoid(x):
    out = np.empty_like(x, dtype=np.float32)
    pos = x >= 0
    out[pos] = 1.0 / (1.0 + np.exp(-x[pos]))
    ex = np.exp(x[~pos])
    out[~pos] = ex / (1.0 + ex)
    return out


def prep_gates(rep_flat, adj_arc, adj_lab, adj_mask_in, adj_mask_loop, mask,
               W_gate_in, b_gate_in, W_gate_self):
    """Host gate path: per-token gate weights with masks folded in."""
    idx = (adj_arc[..., 0].reshape(-1) * L + adj_arc[..., 1].reshape(-1)).astype(np.int64)
    lab = adj_lab.reshape(-1).astype(np.int64)
    g_in = (rep_flat @ np.asarray(W_gate_in, np.float32)[:, 0])[idx] \
        + np.asarray(b_gate_in, np.float32)[lab, 0]
    g_self = rep_flat @ np.asarray(W_gate_self, np.float32)[:, 0]
    m = np.asarray(mask, np.float32).reshape(-1)
    w_in = _sigmoid(g_in) * np.asarray(adj_mask_in, np.float32).reshape(-1) ** 2 * m
    w_self = _sigmoid(g_self) * np.asarray(adj_mask_loop, np.float32).reshape(-1) ** 2 * m
    return idx, lab, w_in, w_self


def prep_core_inputs(c, rep, idx, lab, w_in, w_self, wcat, ball,
                     ntiles: int = NTILES, lab_bias: bool = False):
    """Build the per-core in_map (host-side shard + layout prep)."""
    tok = ntiles * TILE_T
    lo = c * SPC * L
    rep_s = np.ascontiguousarray(rep[c * SPC:(c + 1) * SPC]).reshape(SPC * L, DIN)[:tok]
    x = rep_s.reshape(ntiles, TILE_T, KC, 128)              # [i, t, kc, k]
    repT = np.ascontiguousarray(x.transpose(3, 0, 2, 1).astype(NP_MM))  # [k, i, kc, t]

    idx_local = idx[lo:lo + tok] - lo
    t_all = np.arange(tok)
    if idx_local.min() < 0 or idx_local.max() >= tok or np.any(idx_local // TILE_T != t_all // TILE_T):
        raise ValueError("head gather escapes its 128-token tile; unsupported input structure")

    w_in_s = w_in[lo:lo + tok].astype(NP_MM)
    scatW = np.zeros((TILE_T, ntiles, TILE_T), NP_MM)
    scatW[idx_local % TILE_T, t_all // TILE_T, t_all % TILE_T] = w_in_s

    aux = np.empty((128, ntiles + 1), np.float32)
    aux[:, :ntiles] = w_self[lo:lo + tok].reshape(ntiles, TILE_T).T
    aux[:, ntiles] = 1.0

    in_map = {"repT": repT, "scatW": scatW, "wcat": wcat, "aux": aux}
    if lab_bias:
        lab_s = lab[lo:lo + tok]
        scatL = np.zeros((NREL, ntiles, TILE_T), NP_MM)
        scatL[lab_s, t_all // TILE_T, t_all % TILE_T] = w_in_s
        in_map["scatL"] = scatL
        in_map["ball"] = ball
    return in_map


def prep_shared(W_in, b_in, W_self):
    wcat = np.concatenate([np.asarray(W_in, np.float32),
                           np.asarray(W_self, np.float32)], axis=1)
    wcat = np.ascontiguousarray(
        wcat.reshape(KC, 128, 2 * DOUT).transpose(1, 0, 2).astype(NP_MM))
    ball = np.ascontiguousarray(np.asarray(b_in, np.float32).astype(NP_MM))
    return wcat, ball


def unshard_out(raw):
    """[128, ntiles, DOUT] fp16 partition-major -> [SPC, L, DOUT] fp32."""
    return raw.transpose(1, 0, 2).astype(np.float32).reshape(SPC, L, DOUT)


_NC_CACHE = {}


def get_nc(lab_bias: bool):
    if lab_bias not in _NC_CACHE:
        _NC_CACHE[lab_bias] = build_nc(lab_bias=lab_bias)
    return _NC_CACHE[lab_bias]


def kernel(rep, adj_mask_in, adj_mask_loop, mask, W_in, b_in, W_gate_in,
           b_gate_in, W_self, W_gate_self, adj_arc_in, adj_lab_in):
    rep = np.asarray(rep, dtype=np.float32)
    b_in = np.asarray(b_in, dtype=np.float32)
    lab_bias = bool(np.any(b_in != 0.0))
    rep_flat = rep.reshape(BNK * L, DIN)
    idx, lab, w_in, w_self = prep_gates(
        rep_flat, np.asarray(adj_arc_in), np.asarray(adj_lab_in),
        adj_mask_in, adj_mask_loop, mask, W_gate_in, b_gate_in, W_gate_self)
    wcat, ball = prep_shared(W_in, b_in, W_self)
    in_maps = [
        prep_core_inputs(c, rep, idx, lab, w_in, w_self, wcat, ball, lab_bias=lab_bias)
        for c in range(NCORES)
    ]

    nc = get_nc(lab_bias)
    res = bass_utils.run_bass_kernel_spmd(nc, in_maps, core_ids=list(range(NCORES)))
    out = np.concatenate([unshard_out(r["out"]) for r in res.results], axis=0)
    return out


# revision 21
# speedup vs baseline: 1.0151x; 1.0151x over previous
"""GCNN message-passing layer on 8 Trainium2 NeuronCores (Bass/Tile).

Math (per token m, all within one sentence of L=64 tokens):
    in_pot[m]  = (rep @ W_in)[head(m)] + b_in[lab(m)]
    in_gate[m] = (rep @ W_gate_in)[head(m)] + b_gate_in[lab(m)]
    self_pot   = rep @ W_self ; self_gate = rep @ W_gate_self
    w_d = sigmoid(gate_d) * msoft_d^2
    out = relu(in_pot*w_in + self_pot*w_self) * mask

Sharding: data-parallel over BNK (160 sentences / core). All gathers stay
within a sentence, so shards are independent; weights are replicated.

Device strategy per 128-token tile (2 sentences):
  - The gate paths (rep @ W_gate_*, 0.2% of the FLOPs) run on the host;
    sigmoid(gate)*msoft^2*mask folds into the one-hot scatter values (w_in
    side) and into a per-token aux vector (w_self side). The device never
    computes gates, sigmoids, or masks.
  - rep arrives host-pretransposed (fp16) so DIN sits on partitions. One
    512-column moving operand [W_in | W_self] turns the two projections
    into 4 accumulating matmuls per tile (512-cycle streams fully hide
    each LDWEIGHTS).
  - The within-tile head gather is a matmul with a host-built one-hot
    scatter matrix whose nonzeros carry w_in; it is software-pipelined one
    tile behind the projections so the in-order tensor queue never waits
    on the PSUM->fp16 cast feeding it. (A relation-bias matmul joins the
    same accumulation only when b_in != 0; setup_inputs has b_in == 0.)
  - Tail per tile: one ACT op casts the whole [in_pot|self_pot] PSUM bank
    to fp16 (gather src + self operand), then one fused custom-DVE op
    emits relu(in_pot_gathered + w_self*self_pot) straight to fp16.
  - Output stays partition-major in DRAM ([128, ntiles, dout]) so the DMA
    moves 4KB-contiguous runs; the host de-interleaves.
  - Startup: ~32 throwaway matmuls release the PE HAM clock gate while
    the first DMAs land; wcat rides the Scalar HWDGE queue concurrently
    with rep on the SP queue, and the first four tiles' rep arrives as
    single-tile DMAs, so real matmuls start ~9 us in, still warm.
  - Outputs ride the GpSimd SWDGE queue except the last batch, which goes
    on the Scalar HWDGE queue so the end-of-kernel SWDGE drain finds an
    idle queue.
"""

import numpy as np

import concourse.bass as bass
import concourse.dve_ops as dve_ops
import concourse.mybir as mybir
import concourse.tile as tile
from concourse import bacc, bass_utils
from concourse.dve_spec import C0, C1, Spec, Src0, Src1, lower as dve_lower, relu as dve_relu
from concourse.dve_uop import DveOpSpec


def _register_gated_relu_op():
    """Register a fused custom-DVE op: out = relu(in0*s0 + in1*s1)."""
    name = "GCNN_GATED_RELU_ANT"
    for op in dve_ops.OPS:
        if op.name == name:
            return op
    spec = Spec(
        body=dve_relu(Src0 * C0 + Src1 * C1),
        reference=lambda in0, in1, s0, s1, imm2: np.maximum(
            np.nan_to_num(in0.astype(np.float32) * s0 + in1 * s1,
                          nan=0.0, posinf=np.inf, neginf=-np.inf), 0.0),
    )
    row = dve_ops._CUSTOM_DVE_ROW_BASE + len(dve_ops.OPS)
    dve_ops._SUB_OPCODE_FOR_NAME[name] = row
    shas = {}
    for ver in ("v3", "v4"):
        uops = dve_lower(spec, ver=ver)
        shas[ver] = DveOpSpec(name=name, opcode=row, uops=uops, rd1_en=True).sha(ver)
    op = dve_ops.DveOp(name, spec, subdim=False, uops_sha=shas)
    dve_ops.OPS.append(op)
    dve_ops.CUSTOM_DVE_SPECS[name] = spec
    return op


GATED_RELU = _register_gated_relu_op()

BNK, L, DIN, DOUT, NREL = 1280, 64, 512, 256, 40
NCORES = 8
SPC = BNK // NCORES          # sentences per core
TOK = SPC * L                # tokens per core (10240)
TILE_T = 128                 # tokens per device tile
KC = DIN // 128              # K chunks (4)
NTILES = TOK // TILE_T       # 80
OGROUP = 4                   # tiles per output DMA batch
NWARM = 48                   # HAM warmup matmuls

F32 = mybir.dt.float32
F16 = mybir.dt.float16
NP_MM = np.float16
AF = mybir.ActivationFunctionType


def _in_groups(ntiles):
    """Input DMA batching: single tiles first (fast start), then fours."""
    gs = [(0, 1), (1, 1), (2, 1), (3, 1)]
    i = 4
    while i < ntiles:
        sz = min(4, ntiles - i)
        gs.append((i, sz))
        i += sz
    return gs


def build_nc(ntiles: int = NTILES, lab_bias: bool = False):
    """Build the per-core Bass program (same program on all cores).

    lab_bias=True adds the relation-bias matmul (needed only when b_in is
    nonzero; setup_inputs always produces b_in == 0).
    """
    assert ntiles % OGROUP == 0
    nc = bacc.Bacc("TRN2", target_bir_lowering=False, debug=False)

    # --- DRAM I/O (flat, partition-major; sliced per DMA batch) ---------
    repT_d = nc.dram_tensor("repT", [128, ntiles, KC, TILE_T], F16, kind="ExternalInput")
    scatW_d = nc.dram_tensor("scatW", [TILE_T, ntiles, TILE_T], F16, kind="ExternalInput")
    if lab_bias:
        scatL_d = nc.dram_tensor("scatL", [NREL, ntiles, TILE_T], F16, kind="ExternalInput")
        ball_d = nc.dram_tensor("ball", [NREL, DOUT], F16, kind="ExternalInput")
    wcat_d = nc.dram_tensor("wcat", [128, KC, 2 * DOUT], F16, kind="ExternalInput")
    # aux[:, i] = w_self for tile i; last column is the constant 1.0
    aux_d = nc.dram_tensor("aux", [128, ntiles + 1], F32, kind="ExternalInput")
    # partition-major output: [p, tile, dout]; host de-interleaves
    out_d = nc.dram_tensor("out", [TILE_T, ntiles, DOUT], F16, kind="ExternalOutput")

    groups = _in_groups(ntiles)
    with tile.TileContext(nc) as tc:
        with (
            tc.tile_pool(name="const", bufs=1) as const_pool,
            tc.tile_pool(name="rep", bufs=6) as rep_pool,
            tc.tile_pool(name="scat", bufs=5) as scat_pool,
            tc.tile_pool(name="src", bufs=3) as src_pool,
            tc.tile_pool(name="out", bufs=3) as out_pool,
            tc.tile_pool(name="psum", bufs=3, space="PSUM") as psum_pool,
            tc.tile_pool(name="psum2", bufs=3, space="PSUM") as psum2_pool,
            tc.tile_pool(name="psumw", bufs=1, space="PSUM") as psumw_pool,
        ):
            # --- PE warmup: release the HAM clock gate while DMAs land ---
            wz = const_pool.tile([128, 16], F16)
            nc.gpsimd.memset(wz[:], 0.0)
            wp = psumw_pool.tile([16, 16], F32, tag="warm")
            for _ in range(NWARM):
                nc.tensor.matmul(wp[:], wz[:, 0:16], wz[:], start=True, stop=True)

            # wcat on the Scalar HWDGE queue, concurrent with rep on SP;
            # per-kc slices: the first matmul gates on 128KB, not 512KB
            # (early DMA bandwidth is far below the steady 350GB/s)
            wcat_sb = [const_pool.tile([128, 2 * DOUT], F16, tag=f"wcat{kc}",
                                       name=f"wcat{kc}")
                       for kc in range(KC)]
            for kc in range(KC):
                nc.scalar.dma_start(wcat_sb[kc][:], wcat_d[:, kc, :])
            aux_sb = const_pool.tile([128, ntiles + 1], F32)
            ones = aux_sb[:, ntiles:ntiles + 1]
            ball_sb = const_pool.tile([NREL, DOUT], F16) if lab_bias else None

            pend = None          # (i, src, o_sb, oslot, scat_sb, sslot, scatl_sb)
            pend_out = None      # (ostart, o_sb) awaiting its batched output DMA

            def flush_tail():
                nonlocal pend, pend_out
                if pend is None:
                    return
                i, src, o_sb, oslot, scat_sb, sslot, scatl_sb = pend
                psum_g = psum2_pool.tile([128, DOUT], F32, tag="pg")
                nc.tensor.matmul(psum_g[:], scat_sb[:, sslot, :], src[:, 0:DOUT],
                                 start=True, stop=not lab_bias)
                if lab_bias:
                    nc.tensor.matmul(psum_g[:], scatl_sb[:, sslot, :], ball_sb[:],
                                     start=False, stop=True)
                nc.vector._custom_dve(GATED_RELU, out=o_sb[:, oslot, :],
                                      in0=psum_g[:], in1=src[:, DOUT:2 * DOUT],
                                      s0=ones, s1=aux_sb[:, i:i + 1])
                pend = None
                if oslot == OGROUP - 1:
                    ostart, osb = pend_out
                    eng = nc.scalar if ostart + OGROUP >= ntiles else nc.gpsimd
                    eng.dma_start(out_d[:, ostart:ostart + OGROUP, :], osb[:])
                    pend_out = None

            o_sb = None
            for gi, (i0, sz) in enumerate(groups):
                rep_sb = rep_pool.tile([128, sz, KC, TILE_T], F16, tag="rep")
                nc.sync.dma_start(rep_sb[:], repT_d[:, i0:i0 + sz, :, :])
                if i0 == 0:
                    # tiles 0-3 share one scatter batch; allocate now (tile 0's
                    # tail references it) but DMA after rep1 so the queue
                    # delivers bytes in consumption order
                    scat_sb = scat_pool.tile([TILE_T, 4, TILE_T], F16, tag="scath")
                    s0 = 0
                    if lab_bias:
                        scatl_sb = scat_pool.tile([NREL, 4, TILE_T], F16, tag="scatl")
                elif i0 == 1:
                    # emitted before flush_tail(tile 0), which reads these
                    nc.sync.dma_start(scat_sb[:], scatW_d[:, 0:4, :])
                    nc.sync.dma_start(aux_sb[:], aux_d[:])
                    if lab_bias:
                        nc.sync.dma_start(scatl_sb[:], scatL_d[:, 0:4, :])
                        nc.sync.dma_start(ball_sb[:], ball_d[:])
                elif i0 >= 4:
                    scat_sb = scat_pool.tile([TILE_T, sz, TILE_T], F16, tag="scath")
                    nc.sync.dma_start(scat_sb[:], scatW_d[:, i0:i0 + sz, :])
                    s0 = i0
                    if lab_bias:
                        scatl_sb = scat_pool.tile([NREL, sz, TILE_T], F16, tag="scatl")
                        nc.sync.dma_start(scatl_sb[:], scatL_d[:, i0:i0 + sz, :])

                for ti in range(sz):
                    i = i0 + ti
                    if i % OGROUP == 0:
                        o_sb = out_pool.tile([128, OGROUP, DOUT], F16)
                    # [in_pot | self_pot] in one PSUM bank via a fused
                    # 512-column moving operand
                    psum_ab = psum_pool.tile([128, 2 * DOUT], F32, tag="pab")
                    for kc in range(KC):
                        nc.tensor.matmul(psum_ab[:], rep_sb[:, ti, kc, :],
                                         wcat_sb[kc][:],
                                         start=kc == 0, stop=kc == KC - 1)
                    src = src_pool.tile([128, 2 * DOUT], F16)
                    nc.scalar.activation(src[:], psum_ab[:], AF.Copy)
                    flush_tail()
                    if i % OGROUP == OGROUP - 1:
                        pend_out = (i - OGROUP + 1, o_sb)
                    pend = (i, src, o_sb, i % OGROUP, scat_sb, i - s0,
                            scatl_sb if lab_bias else None)
            flush_tail()

    nc.compile()
    return nc


def _sigmoid(x):
    out = np.empty_like(x, dtype=np.float32)
    pos = x >= 0
    out[pos] = 1.0 / (1.0 + np.exp(-x[pos]))
    ex = np.exp(x[~pos])
    out[~pos] = ex / (1.0 + ex)
    return out


def prep_gates(rep_flat, adj_arc, adj_lab, adj_mask_in, adj_mask_loop, mask,
               W_gate_in, b_gate_in, W_gate_self):
    """Host gate path: per-token gate weights with masks folded in."""
    idx = (adj_arc[..., 0].reshape(-1) * L + adj_arc[..., 1].reshape(-1)).astype(np.int64)
    lab = adj_lab.reshape(-1).astype(np.int64)
    g_in = (rep_flat @ np.asarray(W_gate_in, np.float32)[:, 0])[idx] \
        + np.asarray(b_gate_in, np.float32)[lab, 0]
    g_self = rep_flat @ np.asarray(W_gate_self, np.float32)[:, 0]
    m = np.asarray(mask, np.float32).reshape(-1)
    w_in = _sigmoid(g_in) * np.asarray(adj_mask_in, np.float32).reshape(-1) ** 2 * m
    w_self = _sigmoid(g_self) * np.asarray(adj_mask_loop, np.float32).reshape(-1) ** 2 * m
    return idx, lab, w_in, w_self


def prep_core_inputs(c, rep, idx, lab, w_in, w_self, wcat, ball,
                     ntiles: int = NTILES, lab_bias: bool = False):
    """Build the per-core in_map (host-side shard + layout prep)."""
    tok = ntiles * TILE_T
    lo = c * SPC * L
    rep_s = np.ascontiguousarray(rep[c * SPC:(c + 1) * SPC]).reshape(SPC * L, DIN)[:tok]
    x = rep_s.reshape(ntiles, TILE_T, KC, 128)              # [i, t, kc, k]
    repT = np.ascontiguousarray(x.transpose(3, 0, 2, 1).astype(NP_MM))  # [k, i, kc, t]

    idx_local = idx[lo:lo + tok] - lo
    t_all = np.arange(tok)
    if idx_local.min() < 0 or idx_local.max() >= tok or np.any(idx_local // TILE_T != t_all // TILE_T):
        raise ValueError("head gather escapes its 128-token tile; unsupported input structure")

    w_in_s = w_in[lo:lo + tok].astype(NP_MM)
    scatW = np.zeros((TILE_T, ntiles, TILE_T), NP_MM)
    scatW[idx_local % TILE_T, t_all // TILE_T, t_all % TILE_T] = w_in_s

    aux = np.empty((128, ntiles + 1), np.float32)
    aux[:, :ntiles] = w_self[lo:lo + tok].reshape(ntiles, TILE_T).T
    aux[:, ntiles] = 1.0

    in_map = {"repT": repT, "scatW": scatW, "wcat": wcat, "aux": aux}
    if lab_bias:
        lab_s = lab[lo:lo + tok]
        scatL = np.zeros((NREL, ntiles, TILE_T), NP_MM)
        scatL[lab_s, t_all // TILE_T, t_all % TILE_T] = w_in_s
        in_map["scatL"] = scatL
        in_map["ball"] = ball
    return in_map


def prep_shared(W_in, b_in, W_self):
    wcat = np.concatenate([np.asarray(W_in, np.float32),
                           np.asarray(W_self, np.float32)], axis=1)
    wcat = np.ascontiguousarray(
        wcat.reshape(KC, 128, 2 * DOUT).transpose(1, 0, 2).astype(NP_MM))
    ball = np.ascontiguousarray(np.asarray(b_in, np.float32).astype(NP_MM))
    return wcat, ball


def unshard_out(raw):
    """[128, ntiles, DOUT] fp16 partition-major -> [SPC, L, DOUT] fp32."""
    return raw.transpose(1, 0, 2).astype(np.float32).reshape(SPC, L, DOUT)


_NC_CACHE = {}


def get_nc(lab_bias: bool):
    if lab_bias not in _NC_CACHE:
        _NC_CACHE[lab_bias] = build_nc(lab_bias=lab_bias)
    return _NC_CACHE[lab_bias]


def kernel(rep, adj_mask_in, adj_mask_loop, mask, W_in, b_in, W_gate_in,
           b_gate_in, W_self, W_gate_self, adj_arc_in, adj_lab_in):
    rep = np.asarray(rep, dtype=np.float32)
    b_in = np.asarray(b_in, dtype=np.float32)
    lab_bias = bool(np.any(b_in != 0.0))
    rep_flat = rep.reshape(BNK * L, DIN)
    idx, lab, w_in, w_self = prep_gates(
        rep_flat, np.asarray(adj_arc_in), np.asarray(adj_lab_in),
        adj_mask_in, adj_mask_loop, mask, W_gate_in, b_gate_in, W_gate_self)
    wcat, ball = prep_shared(W_in, b_in, W_self)
    in_maps = [
        prep_core_inputs(c, rep, idx, lab, w_in, w_self, wcat, ball, lab_bias=lab_bias)
        for c in range(NCORES)
    ]

    nc = get_nc(lab_bias)
    res = bass_utils.run_bass_kernel_spmd(nc, in_maps, core_ids=list(range(NCORES)))
    out = np.concatenate([unshard_out(r["out"]) for r in res.results], axis=0)
    return out


# revision 25
# speedup vs baseline: 1.0152x; 1.0001x over previous
"""GCNN message-passing layer on 8 Trainium2 NeuronCores (Bass/Tile).

Math (per token m, all within one sentence of L=64 tokens):
    in_pot[m]  = (rep @ W_in)[head(m)] + b_in[lab(m)]
    in_gate[m] = (rep @ W_gate_in)[head(m)] + b_gate_in[lab(m)]
    self_pot   = rep @ W_self ; self_gate = rep @ W_gate_self
    w_d = sigmoid(gate_d) * msoft_d^2
    out = relu(in_pot*w_in + self_pot*w_self) * mask

Sharding: data-parallel over BNK (160 sentences / core). All gathers stay
within a sentence, so shards are independent; weights are replicated.

Device strategy per 128-token tile (2 sentences):
  - The gate paths (rep @ W_gate_*, 0.2% of the FLOPs) run on the host;
    sigmoid(gate)*msoft^2*mask folds into the one-hot scatter values (w_in
    side) and into a per-token aux vector (w_self side). The device never
    computes gates, sigmoids, or masks.
  - rep arrives host-pretransposed (fp16) so DIN sits on partitions. One
    512-column moving operand [W_in | W_self] turns the two projections
    into 4 accumulating matmuls per tile (512-cycle streams fully hide
    each LDWEIGHTS).
  - The within-tile head gather is a matmul with a host-built one-hot
    scatter matrix whose nonzeros carry w_in; it is software-pipelined one
    tile behind the projections so the in-order tensor queue never waits
    on the PSUM->fp16 cast feeding it. (A relation-bias matmul joins the
    same accumulation only when b_in != 0; setup_inputs has b_in == 0.)
  - Tail per tile: one ACT op casts the whole [in_pot|self_pot] PSUM bank
    to fp16 (gather src + self operand), then one fused custom-DVE op
    emits relu(in_pot_gathered + w_self*self_pot) straight to fp16.
  - Output stays partition-major in DRAM ([128, ntiles, dout]) so the DMA
    moves 4KB-contiguous runs; the host de-interleaves.
  - Startup: ~32 throwaway matmuls release the PE HAM clock gate while
    the first DMAs land; wcat rides the Scalar HWDGE queue concurrently
    with rep on the SP queue, and the first four tiles' rep arrives as
    single-tile DMAs, so real matmuls start ~9 us in, still warm.
  - Outputs ride the GpSimd SWDGE queue except the last batch, which goes
    on the Scalar HWDGE queue so the end-of-kernel SWDGE drain finds an
    idle queue.
"""

import numpy as np

import concourse.bass as bass
import concourse.dve_ops as dve_ops
import concourse.mybir as mybir
import concourse.tile as tile
from concourse import bacc, bass_utils
from concourse.dve_spec import C0, C1, Spec, Src0, Src1, lower as dve_lower, relu as dve_relu
from concourse.dve_uop import DveOpSpec


def _register_gated_relu_op():
    """Register a fused custom-DVE op: out = relu(in0*s0 + in1*s1)."""
    name = "GCNN_GATED_RELU_ANT"
    for op in dve_ops.OPS:
        if op.name == name:
            return op
    spec = Spec(
        body=dve_relu(Src0 * C0 + Src1 * C1),
        reference=lambda in0, in1, s0, s1, imm2: np.maximum(
            np.nan_to_num(in0.astype(np.float32) * s0 + in1 * s1,
                          nan=0.0, posinf=np.inf, neginf=-np.inf), 0.0),
    )
    row = dve_ops._CUSTOM_DVE_ROW_BASE + len(dve_ops.OPS)
    dve_ops._SUB_OPCODE_FOR_NAME[name] = row
    shas = {}
    for ver in ("v3", "v4"):
        uops = dve_lower(spec, ver=ver)
        shas[ver] = DveOpSpec(name=name, opcode=row, uops=uops, rd1_en=True).sha(ver)
    op = dve_ops.DveOp(name, spec, subdim=False, uops_sha=shas)
    dve_ops.OPS.append(op)
    dve_ops.CUSTOM_DVE_SPECS[name] = spec
    return op


GATED_RELU = _register_gated_relu_op()

BNK, L, DIN, DOUT, NREL = 1280, 64, 512, 256, 40
NCORES = 8
SPC = BNK // NCORES          # sentences per core
TOK = SPC * L                # tokens per core (10240)
TILE_T = 128                 # tokens per device tile
KC = DIN // 128              # K chunks (4)
NTILES = TOK // TILE_T       # 80
OGROUP = 4                   # tiles per output DMA batch
NWARM = 40                   # HAM warmup matmuls (short)
NWARMB = 12                  # HAM warmup matmuls (128-col, bridge the DMA wait)

F32 = mybir.dt.float32
F16 = mybir.dt.float16
NP_MM = np.float16
AF = mybir.ActivationFunctionType


def _in_groups(ntiles):
    """Input DMA batching: single tiles first (fast start), then fours."""
    gs = [(0, 1), (1, 1), (2, 1), (3, 1)]
    i = 4
    while i < ntiles:
        sz = min(4, ntiles - i)
        gs.append((i, sz))
        i += sz
    return gs


def build_nc(ntiles: int = NTILES, lab_bias: bool = False):
    """Build the per-core Bass program (same program on all cores).

    lab_bias=True adds the relation-bias matmul (needed only when b_in is
    nonzero; setup_inputs always produces b_in == 0).
    """
    assert ntiles % OGROUP == 0
    nc = bacc.Bacc("TRN2", target_bir_lowering=False, debug=False)

    # --- DRAM I/O (flat, partition-major; sliced per DMA batch) ---------
    repT_d = nc.dram_tensor("repT", [128, ntiles, KC, TILE_T], F16, kind="ExternalInput")
    scatW_d = nc.dram_tensor("scatW", [TILE_T, ntiles, TILE_T], F16, kind="ExternalInput")
    if lab_bias:
        scatL_d = nc.dram_tensor("scatL", [NREL, ntiles, TILE_T], F16, kind="ExternalInput")
        ball_d = nc.dram_tensor("ball", [NREL, DOUT], F16, kind="ExternalInput")
    wcat_d = nc.dram_tensor("wcat", [128, KC, 2 * DOUT], F16, kind="ExternalInput")
    # aux[:, i] = w_self for tile i; last column is the constant 1.0
    aux_d = nc.dram_tensor("aux", [128, ntiles + 1], F32, kind="ExternalInput")
    # partition-major output: [p, tile, dout]; host de-interleaves
    out_d = nc.dram_tensor("out", [TILE_T, ntiles, DOUT], F16, kind="ExternalOutput")

    groups = _in_groups(ntiles)
    with tile.TileContext(nc) as tc:
        with (
            tc.tile_pool(name="const", bufs=1) as const_pool,
            tc.tile_pool(name="rep", bufs=6) as rep_pool,
            tc.tile_pool(name="scat", bufs=5) as scat_pool,
            tc.tile_pool(name="src", bufs=3) as src_pool,
            tc.tile_pool(name="out", bufs=3) as out_pool,
            tc.tile_pool(name="psum", bufs=3, space="PSUM") as psum_pool,
            tc.tile_pool(name="psum2", bufs=3, space="PSUM") as psum2_pool,
            tc.tile_pool(name="psumw", bufs=1, space="PSUM") as psumw_pool,
        ):
            # --- PE warmup: release the HAM clock gate while DMAs land.
            # The warm MID-window re-throttles after ~1.7us of PE idle at
            # 2.4GHz, so longer 128-col matmuls bridge until real data.
            wz = const_pool.tile([128, 128], F16)
            nc.gpsimd.memset(wz[:], 0.0)
            wp = psumw_pool.tile([128, 128], F32, tag="warm")
            for _ in range(NWARM):
                nc.tensor.matmul(wp[0:16, 0:16], wz[:, 0:16], wz[:, 0:16],
                                 start=True, stop=True)
            for _ in range(NWARMB):
                nc.tensor.matmul(wp[:], wz[:], wz[:], start=True, stop=True)

            # wcat on the Scalar HWDGE queue, concurrent with rep on SP;
            # per-kc slices: the first matmul gates on 128KB, not 512KB
            # (early DMA bandwidth is far below the steady 350GB/s)
            wcat_sb = [const_pool.tile([128, 2 * DOUT], F16, tag=f"wcat{kc}",
                                       name=f"wcat{kc}")
                       for kc in range(KC)]
            for kc in range(KC):
                nc.scalar.dma_start(wcat_sb[kc][:], wcat_d[:, kc, :])
            aux_sb = const_pool.tile([128, ntiles + 1], F32)
            ones = aux_sb[:, ntiles:ntiles + 1]
            ball_sb = const_pool.tile([NREL, DOUT], F16) if lab_bias else None

            pend = None          # (i, src, o_sb, oslot, scat_sb, sslot, scatl_sb)
            pend_out = None      # (ostart, o_sb) awaiting its batched output DMA

            def flush_tail():
                nonlocal pend, pend_out
                if pend is None:
                    return
                i, src, o_sb, oslot, scat_sb, sslot, scatl_sb = pend
                psum_g = psum2_pool.tile([128, DOUT], F32, tag="pg")
                nc.tensor.matmul(psum_g[:], scat_sb[:, sslot, :], src[:, 0:DOUT],
                                 start=True, stop=not lab_bias)
                if lab_bias:
                    nc.tensor.matmul(psum_g[:], scatl_sb[:, sslot, :], ball_sb[:],
                                     start=False, stop=True)
                nc.vector._custom_dve(GATED_RELU, out=o_sb[:, oslot, :],
                                      in0=psum_g[:], in1=src[:, DOUT:2 * DOUT],
                                      s0=ones, s1=aux_sb[:, i:i + 1])
                pend = None
                if i == ntiles - 2:
                    # final group: ship the first three tiles early so the
                    # very last transfer is a single 64KB tile
                    nc.scalar.dma_start(out_d[:, ntiles - OGROUP:ntiles - 1, :],
                                        o_sb[:, 0:OGROUP - 1, :])
                elif i == ntiles - 1:
                    nc.scalar.dma_start(out_d[:, i:i + 1, :], o_sb[:, oslot:oslot + 1, :])
                    pend_out = None
                elif oslot == OGROUP - 1:
                    ostart, osb = pend_out
                    nc.gpsimd.dma_start(out_d[:, ostart:ostart + OGROUP, :], osb[:])
                    pend_out = None

            o_sb = None
            for gi, (i0, sz) in enumerate(groups):
                rep_sb = rep_pool.tile([128, sz, KC, TILE_T], F16, tag="rep")
                nc.sync.dma_start(rep_sb[:], repT_d[:, i0:i0 + sz, :, :])
                if i0 == 0:
                    # tiles 0-3 share one scatter batch; allocate now (tile 0's
                    # tail references it) but DMA after rep1 so the queue
                    # delivers bytes in consumption order
                    scat_sb = scat_pool.tile([TILE_T, 4, TILE_T], F16, tag="scath")
                    s0 = 0
                    if lab_bias:
                        scatl_sb = scat_pool.tile([NREL, 4, TILE_T], F16, tag="scatl")
                elif i0 == 1:
                    # emitted before flush_tail(tile 0), which reads these
                    nc.sync.dma_start(scat_sb[:], scatW_d[:, 0:4, :])
                    nc.sync.dma_start(aux_sb[:], aux_d[:])
                    if lab_bias:
                        nc.sync.dma_start(scatl_sb[:], scatL_d[:, 0:4, :])
                        nc.sync.dma_start(ball_sb[:], ball_d[:])
                elif i0 >= 4:
                    scat_sb = scat_pool.tile([TILE_T, sz, TILE_T], F16, tag="scath")
                    nc.sync.dma_start(scat_sb[:], scatW_d[:, i0:i0 + sz, :])
                    s0 = i0
                    if lab_bias:
                        scatl_sb = scat_pool.tile([NREL, sz, TILE_T], F16, tag="scatl")
                        nc.sync.dma_start(scatl_sb[:], scatL_d[:, i0:i0 + sz, :])

                for ti in range(sz):
                    i = i0 + ti
                    if i % OGROUP == 0:
                        o_sb = out_pool.tile([128, OGROUP, DOUT], F16)
                    # [in_pot | self_pot] in one PSUM bank via a fused
                    # 512-column moving operand
                    psum_ab = psum_pool.tile([128, 2 * DOUT], F32, tag="pab")
                    for kc in range(KC):
                        nc.tensor.matmul(psum_ab[:], rep_sb[:, ti, kc, :],
                                         wcat_sb[kc][:],
                                         start=kc == 0, stop=kc == KC - 1)
                    src = src_pool.tile([128, 2 * DOUT], F16)
                    nc.scalar.activation(src[:], psum_ab[:], AF.Copy)
                    flush_tail()
                    if i % OGROUP == OGROUP - 1:
                        pend_out = (i - OGROUP + 1, o_sb)
                    pend = (i, src, o_sb, i % OGROUP, scat_sb, i - s0,
                            scatl_sb if lab_bias else None)
            flush_tail()

    nc.compile()
    return nc


def _sigmoid(x):
    out = np.empty_like(x, dtype=np.float32)
    pos = x >= 0
    out[pos] = 1.0 / (1.0 + np.exp(-x[pos]))
    ex = np.exp(x[~pos])
    out[~pos] = ex / (1.0 + ex)
    return out


def prep_gates(rep_flat, adj_arc, adj_lab, adj_mask_in, adj_mask_loop, mask,
               W_gate_in, b_gate_in, W_gate_self):
    """Host gate path: per-token gate weights with masks folded in."""
    idx = (adj_arc[..., 0].reshape(-1) * L + adj_arc[..., 1].reshape(-1)).astype(np.int64)
    lab = adj_lab.reshape(-1).astype(np.int64)
    g_in = (rep_flat @ np.asarray(W_gate_in, np.float32)[:, 0])[idx] \
        + np.asarray(b_gate_in, np.float32)[lab, 0]
    g_self = rep_flat @ np.asarray(W_gate_self, np.float32)[:, 0]
    m = np.asarray(mask, np.float32).reshape(-1)
    w_in = _sigmoid(g_in) * np.asarray(adj_mask_in, np.float32).reshape(-1) ** 2 * m
    w_self = _sigmoid(g_self) * np.asarray(adj_mask_loop, np.float32).reshape(-1) ** 2 * m
    return idx, lab, w_in, w_self


def prep_core_inputs(c, rep, idx, lab, w_in, w_self, wcat, ball,
                     ntiles: int = NTILES, lab_bias: bool = False):
    """Build the per-core in_map (host-side shard + layout prep)."""
    tok = ntiles * TILE_T
    lo = c * SPC * L
    rep_s = np.ascontiguousarray(rep[c * SPC:(c + 1) * SPC]).reshape(SPC * L, DIN)[:tok]
    x = rep_s.reshape(ntiles, TILE_T, KC, 128)              # [i, t, kc, k]
    repT = np.ascontiguousarray(x.transpose(3, 0, 2, 1).astype(NP_MM))  # [k, i, kc, t]

    idx_local = idx[lo:lo + tok] - lo
    t_all = np.arange(tok)
    if idx_local.min() < 0 or idx_local.max() >= tok or np.any(idx_local // TILE_T != t_all // TILE_T):
        raise ValueError("head gather escapes its 128-token tile; unsupported input structure")

    w_in_s = w_in[lo:lo + tok].astype(NP_MM)
    scatW = np.zeros((TILE_T, ntiles, TILE_T), NP_MM)
    scatW[idx_local % TILE_T, t_all // TILE_T, t_all % TILE_T] = w_in_s

    aux = np.empty((128, ntiles + 1), np.float32)
    aux[:, :ntiles] = w_self[lo:lo + tok].reshape(ntiles, TILE_T).T
    aux[:, ntiles] = 1.0

    in_map = {"repT": repT, "scatW": scatW, "wcat": wcat, "aux": aux}
    if lab_bias:
        lab_s = lab[lo:lo + tok]
        scatL = np.zeros((NREL, ntiles, TILE_T), NP_MM)
        scatL[lab_s, t_all // TILE_T, t_all % TILE_T] = w_in_s
        in_map["scatL"] = scatL
        in_map["ball"] = ball
    return in_map


def prep_shared(W_in, b_in, W_self):
    wcat = np.concatenate([np.asarray(W_in, np.float32),
                           np.asarray(W_self, np.float32)], axis=1)
    wcat = np.ascontiguousarray(
        wcat.reshape(KC, 128, 2 * DOUT).transpose(1, 0, 2).astype(NP_MM))
    ball = np.ascontiguousarray(np.asarray(b_in, np.float32).astype(NP_MM))
    return wcat, ball


def unshard_out(raw):
    """[128, ntiles, DOUT] fp16 partition-major -> [SPC, L, DOUT] fp32."""
    return raw.transpose(1, 0, 2).astype(np.float32).reshape(SPC, L, DOUT)


_NC_CACHE = {}


def get_nc(lab_bias: bool):
    if lab_bias not in _NC_CACHE:
        _NC_CACHE[lab_bias] = build_nc(lab_bias=lab_bias)
    return _NC_CACHE[lab_bias]


def kernel(rep, adj_mask_in, adj_mask_loop, mask, W_in, b_in, W_gate_in,
           b_gate_in, W_self, W_gate_self, adj_arc_in, adj_lab_in):
    rep = np.asarray(rep, dtype=np.float32)
    b_in = np.asarray(b_in, dtype=np.float32)
    lab_bias = bool(np.any(b_in != 0.0))
    rep_flat = rep.reshape(BNK * L, DIN)
    idx, lab, w_in, w_self = prep_gates(
        rep_flat, np.asarray(adj_arc_in), np.asarray(adj_lab_in),
        adj_mask_in, adj_mask_loop, mask, W_gate_in, b_gate_in, W_gate_self)
    wcat, ball = prep_shared(W_in, b_in, W_self)
    in_maps = [
        prep_core_inputs(c, rep, idx, lab, w_in, w_self, wcat, ball, lab_bias=lab_bias)
        for c in range(NCORES)
    ]

    nc = get_nc(lab_bias)
    res = bass_utils.run_bass_kernel_spmd(nc, in_maps, core_ids=list(range(NCORES)))
    out = np.concatenate([unshard_out(r["out"]) for r in res.results], axis=0)
    return out
